# revision 19
# baseline (speedup 1.0000x reference)
"""Trainium2 Bass kernel for routed-token transformer block (moe_routing).

Strategy (8 NeuronCores):
  - Batch-parallel transformer block: core i owns sample i (B=8).
  - The dominant cost is the router k_predictor matvec x_flat @ kp_w1 with
    kp_w1 [S*D, 512] = 1GB fp32. Its contraction dim is sharded 8-ways:
    core i reads rows [i*65536, (i+1)*65536) once (128MB/core), computes
    partial sums for ALL samples [8, 512], and a ReduceScatter(add) hands
    core i the finished row for its own sample (16KB collective).
  - Attention runs in transposed-activation layout (features on partitions)
    so q/k slices feed matmuls directly. The key mask enters as a
    per-key scale on an augmented V (exp(score+m) == exp(score)*exp(m)),
    so the whole scores+exp pipeline is independent of the router and
    overlaps the kp_w1 DMA stream. An extra ones-column in the augmented V
    yields softmax denominators for free.
"""

import numpy as np

import concourse.bass as bass
import concourse.bacc as bacc
import concourse.mybir as mybir
import concourse.tile as tile
from concourse.masks import make_identity

F32 = mybir.dt.float32
AF = mybir.ActivationFunctionType
OP = mybir.AluOpType

FULL_CFG = dict(S=2048, D=256, H=8, MLP=1024, KH=512, B=8)
GELU_C = 0.7978845608028654  # sqrt(2/pi)
NEG_EPS = 1e-30


def _bcast_ap(handle, p):
    """DRAM AP broadcast over p partitions (partition step 0)."""
    ap = handle.ap()
    return bass.AP(tensor=ap.tensor, offset=ap.offset,
                   ap=[[0, p]] + [list(x) for x in ap.ap[1:]])


def build_bass(cfg):
    S, D, H, MLP_, KH, B = (cfg[k] for k in ("S", "D", "H", "MLP", "KH", "B"))
    P = 128
    HD = D // H
    assert HD == 32
    NT = S // P                      # token tiles
    DT = D // P                      # feature tiles of D
    KSLICE = S * D // B              # router contraction rows per core
    KT = KSLICE // P                 # router K-tiles
    HPT = P // HD                    # heads per 128-feature tile
    CH = min(1024, S)                # attention sq chunk (psum free size)
    NSQC = S // CH
    MMN = min(512, CH)               # matmul free-dim piece
    NPC = CH // MMN
    SQC = min(256, S)                # mlp sq chunk
    NMC = S // SQC
    TPC = SQC // P                   # token tiles per mlp chunk
    MT = MLP_ // P                   # mlp feature tiles
    QKM = 2 * D // P                 # q+k feature tiles
    W1COLS = B + KH
    NQT = (H + 2) // 3               # q/k head-tiles (3 heads per tile @ bases 0/32/64)
    PSW = max(CH, TPC * 512, 512)    # shared psum slot width (bank-aligned tr slots)

    nc = bacc.Bacc()

    # ---------------- DRAM parameters ----------------
    dp = lambda name, shape: nc.declare_dram_parameter(name, list(shape), F32, isOutput=False)
    x_d = dp("x_own", (S, D))
    w1x_d = dp("w1x", (P, KT * W1COLS))
    amask_d = dp("amask_t", (P, NT))
    wpw_d = dp("wp_w_row", (1, D))
    wpb_d = dp("wp_b", (1, 1))
    kpb1_d = dp("kp_b1_row", (1, KH))
    kpw2_d = dp("kp_w2_row", (1, KH))
    kpb2_d = dp("kp_b2", (1, 1))
    ln1g_d = dp("ln1_g_row", (1, D))
    ln1b_d = dp("ln1_b_row", (1, D))
    wqk_d = dp("wqk", (D, 2 * D))
    bqk_d = dp("bqk_col", (2 * D, 1))
    wv_d = dp("wv", (D, D))
    bv_d = dp("bv_row", (1, D))
    wo_d = dp("wo", (D, D))
    bo_d = dp("bo_row", (1, D))
    ln2g_d = dp("ln2_g_row", (1, D))
    ln2b_d = dp("ln2_b_row", (1, D))
    wm1_d = dp("wm1", (D, MLP_))
    bm1_d = dp("bm1_col", (MLP_, 1))
    wm2h_d = dp("wm2_half", (MLP_, D))   # pre-scaled by 0.5 on host (gelu's 0.5)
    bm2_d = dp("bm2_row", (1, D))
    E_d = [dp(f"Emat{ht}", (H, P)) for ht in range(D // P)]
    iota_d = dp("iota_row", (4, S // 4))
    out_d = nc.declare_dram_parameter("out", [S, D], F32, isOutput=True)

    rs_in = nc.dram_tensor("rs_in", [B, KH], F32)
    rs_out = nc.dram_tensor("rs_out", [1, KH], F32)

    with tile.TileContext(nc) as tc:
        with (
            tc.tile_pool(name="singles", bufs=1) as SG,
            tc.tile_pool(name="wA", bufs=max(DT * MT, DT * QKM)) as WA,
            tc.tile_pool(name="wB", bufs=max(MT, 2 * DT)) as WB,
            tc.tile_pool(name="w1chunk", bufs=4) as W1P,
            tc.tile_pool(name="xin", bufs=3) as XP,
            tc.tile_pool(name="tmp", bufs=3) as TMP,
            tc.tile_pool(name="hT", bufs=DT) as HTP,
            tc.tile_pool(name="qk", bufs=1) as QKP,
            tc.tile_pool(name="vaug", bufs=NT) as VAP,
            tc.tile_pool(name="expT", bufs=2) as EXP,
            tc.tile_pool(name="attnT", bufs=1) as ATP,
            tc.tile_pool(name="m1", bufs=2) as M1P,
            tc.tile_pool(name="acts", bufs=NT) as APL,
            tc.tile_pool(name="small", bufs=1) as SM,
            tc.tile_pool(name="ps", bufs=4, space="PSUM") as PS,
        ):
            pst = lambda pp=P: PS.tile([pp, PSW], F32, tag="ps", name="ps")
            # ---------------- constants ----------------
            ident = SG.tile([P, P], F32, tag="ident")
            make_identity(nc, ident)
            eps_t = SG.tile([P, 1], F32, tag="eps")
            nc.vector.memset(eps_t, 1e-5)

            def bload(handle, tag):
                t = SG.tile([P, handle.ap().ap[-1][1]], F32, tag=tag, name=tag)
                nc.sync.dma_start(out=t, in_=_bcast_ap(handle, P))
                return t

            ln1g_b = bload(ln1g_d, "ln1g")
            ln1b_b = bload(ln1b_d, "ln1b")
            ln2g_b = bload(ln2g_d, "ln2g")
            ln2b_b = bload(ln2b_d, "ln2b")
            wpw_b = bload(wpw_d, "wpw")
            wpb_b = bload(wpb_d, "wpb")
            bv_b = bload(bv_d, "bv")
            bo_b = bload(bo_d, "bo")
            bm2_b = bload(bm2_d, "bm2")

            bqk_sb = []
            for mt in range(QKM):
                t = SG.tile([P, 1], F32, tag=f"bqk{mt}", name=f"bqk{mt}")
                nc.sync.dma_start(out=t, in_=bqk_d.ap()[mt * P:(mt + 1) * P, :])
                bqk_sb.append(t)
            bm1_sb = []
            for mt in range(MT):
                t = SG.tile([P, 1], F32, tag=f"bm1{mt}", name=f"bm1{mt}")
                nc.sync.dma_start(out=t, in_=bm1_d.ap()[mt * P:(mt + 1) * P, :])
                bm1_sb.append(t)
            kpb1_sb = SG.tile([1, KH], F32, tag="kpb1")
            nc.sync.dma_start(out=kpb1_sb, in_=kpb1_d.ap())
            kpw2_sb = SG.tile([1, KH], F32, tag="kpw2")
            nc.sync.dma_start(out=kpw2_sb, in_=kpw2_d.ap())
            kpb2_sb = SG.tile([1, 1], F32, tag="kpb2")
            nc.sync.dma_start(out=kpb2_sb, in_=kpb2_d.ap())
            ones_row = SG.tile([1, P], F32, tag="ones_row")
            nc.vector.memset(ones_row, 1.0)
            E_sb = []
            for ht in range(DT):
                e = SG.tile([H, P], F32, tag=f"E{ht}", name=f"E{ht}")
                nc.sync.dma_start(out=e, in_=E_d[ht].ap())
                E_sb.append(e)
            amask_sb = SG.tile([P, NT], F32, tag="amask")
            nc.sync.dma_start(out=amask_sb, in_=amask_d.ap())
            iota_sb = SG.tile([4, S // 4], F32, tag="iota")
            nc.sync.dma_start(out=iota_sb, in_=iota_d.ap())
            ones4 = SG.tile([4, 1], F32, tag="ones4")
            nc.vector.memset(ones4, 1.0)

            # ---------------- block weights (qkv/wo; mlp loaded later in same slots) ----
            wqk_sb = {}
            for kt in range(DT):
                for mt in range(QKM):
                    t = WA.tile([P, P], F32, tag="wA", name="wA")
                    nc.sync.dma_start(out=t, in_=wqk_d.ap()[kt * P:(kt + 1) * P, mt * P:(mt + 1) * P])
                    wqk_sb[kt, mt] = t
            wv_sb = {}
            wo_sb = {}
            for kt in range(DT):
                t = WB.tile([P, D], F32, tag="wB", name="wB")
                nc.sync.dma_start(out=t, in_=wv_d.ap()[kt * P:(kt + 1) * P, :])
                wv_sb[kt] = t
                t2 = WB.tile([P, D], F32, tag="wB", name="wB")
                nc.sync.dma_start(out=t2, in_=wo_d.ap()[kt * P:(kt + 1) * P, :])
                wo_sb[kt] = t2

            # ---------------- router stream ----------------
            ps_router = pst(B)
            for kt in range(KT):
                chunk = W1P.tile([P, W1COLS], F32, tag="w1c", name="w1c")
                nc.sync.dma_start(out=chunk, in_=w1x_d.ap()[:, kt * W1COLS:(kt + 1) * W1COLS])
                nc.tensor.matmul(ps_router[:, 0:KH], chunk[:, 0:B], chunk[:, B:W1COLS],
                                 start=(kt == 0), stop=(kt == KT - 1))

            # ---------------- LN1 + h1T + token weights ----------------
            h1T = [HTP.tile([P, S], F32, tag="hT", name=f"h1T{i}") for i in range(DT)]
            weights_sb = SM.tile([P, NT], F32, tag="weights")
            for t in range(NT):
                x_t = XP.tile([P, D], F32, tag="x1", name="x1")
                nc.sync.dma_start(out=x_t, in_=x_d.ap()[t * P:(t + 1) * P, :])
                stats = TMP.tile([P, 6], F32, tag="stats")
                nc.vector.bn_stats(out=stats, in_=x_t)
                mv = TMP.tile([P, 2], F32, tag="mv")
                nc.vector.bn_aggr(out=mv, in_=stats)
                std = TMP.tile([P, 1], F32, tag="std")
                nc.scalar.activation(out=std, in_=mv[:, 1:2], func=AF.Sqrt, bias=eps_t)
                rstd = TMP.tile([P, 1], F32, tag="rstd")
                nc.vector.reciprocal(out=rstd, in_=std)
                h1 = TMP.tile([P, D], F32, tag="h1")
                nc.vector.tensor_scalar(out=h1, in0=x_t, scalar1=mv[:, 0:1], scalar2=rstd,
                                        op0=OP.subtract, op1=OP.mult)
                nc.vector.tensor_mul(h1, h1, ln1g_b)
                nc.vector.tensor_add(h1, h1, ln1b_b)
                scr = TMP.tile([P, D], F32, tag="scr", bufs=1)
                nc.vector.scalar_tensor_tensor(out=scr, in0=x_t, scalar=1.0, in1=wpw_b,
                                               op0=OP.mult, op1=OP.mult,
                                               accum_out=weights_sb[:, t:t + 1])
                for dt_ in range(DT):
                    tp = pst()
                    nc.tensor.transpose(tp[:, 0:P], h1[:, dt_ * P:(dt_ + 1) * P], ident)
                    nc.vector.tensor_copy(h1T[dt_][:, t * P:(t + 1) * P], tp[:, 0:P])
            nc.vector.tensor_scalar_add(weights_sb, weights_sb, wpb_b[:, 0:1])
            expamask = SM.tile([P, NT], F32, tag="expamask")
            nc.scalar.activation(out=expamask, in_=amask_sb, func=AF.Exp)

            # ---------------- qkT (heads packed 3-per-tile @ bases 0/32/64) ----------
            qh_sb = [QKP.tile([P, S], F32, tag=f"qh{j}", name=f"qh{j}") for j in range(NQT)]
            kh_sb = [QKP.tile([P, S], F32, tag=f"kh{j}", name=f"kh{j}") for j in range(NQT)]

            def head_slice(tiles, h):
                b = 32 * (h % 3)
                return tiles[h // 3][b:b + HD, :]

            QN = min(512, S)
            for mt in range(QKM):
                for ncn in range(S // QN):
                    ps = pst()
                    for kt in range(DT):
                        nc.tensor.matmul(ps[:, 0:QN], wqk_sb[kt, mt],
                                         h1T[kt][:, ncn * QN:(ncn + 1) * QN],
                                         start=(kt == 0), stop=(kt == DT - 1))
                    for g in range(HPT):
                        h = (mt % DT) * HPT + g
                        dst = head_slice(kh_sb if mt >= DT else qh_sb, h)
                        nc.vector.tensor_scalar(
                            out=dst[:, ncn * QN:(ncn + 1) * QN],
                            in0=ps[g * HD:(g + 1) * HD, 0:QN],
                            scalar1=bqk_sb[mt][g * HD:(g + 1) * HD, :],
                            scalar2=None, op0=OP.add)

            # ---------------- V (token-major) + ones col ----------------
            v_aug = []
            for t in range(NT):
                ps = pst()
                for kt in range(DT):
                    nc.tensor.matmul(ps[:, 0:D], h1T[kt][:, t * P:(t + 1) * P], wv_sb[kt],
                                     start=(kt == 0), stop=(kt == DT - 1))
                va = VAP.tile([P, H, HD + 1], F32, tag="vaug", name="vaug")
                nc.vector.tensor_add(va[:, :, 0:HD],
                                     ps[:, 0:D].rearrange("p (h d) -> p h d", h=H),
                                     bv_b.rearrange("p (h d) -> p h d", h=H))
                nc.vector.memset(va[:, :, HD:HD + 1], 1.0)
                v_aug.append(va)

            # ---------------- router epilogue -> k -> sel -> m01/wsel ----------------
            r8 = SM.tile([B, KH], F32, tag="r8")
            nc.vector.tensor_copy(r8, ps_router[:, 0:KH])
            nc.sync.dma_start(out=rs_in.ap(), in_=r8)
            nc.gpsimd.collective_compute(
                "ReduceScatter", OP.add,
                ins=[rs_in.ap()], outs=[rs_out.ap()],
                replica_groups=[list(range(B))],
            )
            klr = SM.tile([1, KH], F32, tag="klr")
            nc.sync.dma_start(out=klr, in_=rs_out.ap())
            nc.vector.tensor_add(klr, klr, kpb1_sb)
            nc.vector.scalar_tensor_tensor(out=klr, in0=klr, scalar=0.01, in1=klr,
                                            op0=OP.mult, op1=OP.max)   # leaky_relu
            scr2 = SM.tile([1, KH], F32, tag="scr2")
            kl2 = SM.tile([1, 1], F32, tag="kl2")
            nc.vector.scalar_tensor_tensor(out=scr2, in0=klr, scalar=1.0, in1=kpw2_sb,
                                           op0=OP.mult, op1=OP.mult, accum_out=kl2)
            nc.vector.tensor_add(kl2, kl2, kpb2_sb)
            sg = SM.tile([1, 1], F32, tag="sg")
            nc.scalar.activation(out=sg, in_=kl2, func=AF.Exp, scale=-1.0)
            nc.vector.tensor_scalar_add(sg, sg, 1.0)
            nc.vector.reciprocal(sg, sg)
            kv = SM.tile([1, 1], F32, tag="kv")
            nc.vector.tensor_scalar(out=kv, in0=sg, scalar1=float(S), scalar2=1.0,
                                    op0=OP.mult, op1=OP.max)
            nc.vector.tensor_scalar_min(kv, kv, float(S))
            # k = floor(clip(sig*S,1,S)) == number of j in [1,S] with j <= v
            psv = pst()
            nc.tensor.matmul(psv[0:4, 0:1], ones_row[:, 0:4], kv,
                             start=True, stop=True)
            vb4 = SM.tile([4, 1], F32, tag="vb4")
            nc.vector.tensor_copy(vb4, psv[0:4, 0:1])
            kcmp = SM.tile([4, S // 4], F32, tag="kcmp")
            nc.vector.tensor_single_scalar(out=kcmp, in_=iota_sb, scalar=vb4,
                                           op=OP.is_le)
            cnt4 = SM.tile([4, 1], F32, tag="cnt4")
            nc.vector.tensor_reduce(out=cnt4, in_=kcmp, axis=mybir.AxisListType.X,
                                    op=OP.add)
            psc = pst()
            nc.tensor.matmul(psc[0:1, 0:1], ones4, cnt4, start=True, stop=True)
            nc.vector.tensor_copy(kv, psc[0:1, 0:1])
            psk = pst()
            nc.tensor.matmul(psk[:, 0:1], ones_row, kv, start=True, stop=True)
            kb = SM.tile([P, 1], F32, tag="kb")
            nc.vector.tensor_copy(kb, psk[:, 0:1])
            sel01 = SM.tile([P, NT], F32, tag="sel01")
            nc.vector.tensor_single_scalar(out=sel01, in_=weights_sb, scalar=kb[:, 0:1], op=OP.is_gt)
            m01 = SM.tile([P, NT], F32, tag="m01")
            nc.vector.tensor_mul(m01, sel01, expamask)
            wsel = SM.tile([P, NT], F32, tag="wsel")
            nc.vector.tensor_mul(wsel, weights_sb, sel01)
            for t in range(NT):
                nc.vector.tensor_scalar_mul(v_aug[t], v_aug[t], m01[:, t:t + 1])

            # ---------------- attention ----------------
            scale = 1.0 / float(np.sqrt(HD))
            attnT = [ATP.tile([P, S], F32, tag=f"attnT{ht}", name=f"attnT{ht}") for ht in range(DT)]
            den_sb = SM.tile([H, S], F32, tag="den")
            for h in range(H):
                ht, hr = divmod(h, HPT)
                qT = head_slice(qh_sb, h)
                kT = head_slice(kh_sb, h)
                for sqc in range(NSQC):
                    pv = pst(HD + 1)
                    for skt in range(NT):
                        ps_s = pst()
                        for j in range(NPC):
                            nc.tensor.matmul(ps_s[:, j * MMN:(j + 1) * MMN],
                                             kT[:, skt * P:(skt + 1) * P],
                                             qT[:, sqc * CH + j * MMN:sqc * CH + (j + 1) * MMN],
                                             start=True, stop=True)
                        et = EXP.tile([P, CH], F32, tag="expT", name="expT")
                        nc.scalar.activation(out=et, in_=ps_s[:, 0:CH], func=AF.Exp, scale=scale)
                        for j in range(NPC):
                            nc.tensor.matmul(pv[:, j * MMN:(j + 1) * MMN],
                                             v_aug[skt][:, h, :],
                                             et[:, j * MMN:(j + 1) * MMN],
                                             start=(skt == 0), stop=(skt == NT - 1))
                    nc.vector.tensor_copy(attnT[ht][hr * HD:(hr + 1) * HD, sqc * CH:(sqc + 1) * CH],
                                          pv[0:HD, 0:CH])
                    dstg = TMP.tile([1, CH], F32, tag="dstg", bufs=2)
                    nc.vector.tensor_copy(dstg, pv[HD:HD + 1, 0:CH])
                    nc.sync.dma_start(out=den_sb[h:h + 1, sqc * CH:(sqc + 1) * CH], in_=dstg)
            nc.vector.tensor_scalar_add(den_sb, den_sb, NEG_EPS)
            nc.vector.reciprocal(den_sb, den_sb)
            for ht in range(DT):
                for sqc in range(NSQC):
                    psb_ = pst()
                    for j in range(NPC):
                        nc.tensor.matmul(psb_[:, j * MMN:(j + 1) * MMN], E_sb[ht],
                                         den_sb[:, sqc * CH + j * MMN:sqc * CH + (j + 1) * MMN],
                                         start=True, stop=True)
                    sl = attnT[ht][:, sqc * CH:(sqc + 1) * CH]
                    nc.vector.tensor_mul(sl, sl, psb_[:, 0:CH])

            # ---------------- a = x + attn@wo + bo ; LN2 -> h2T ----------------
            h2T = [HTP.tile([P, S], F32, tag="hT", name=f"h2T{i}") for i in range(DT)]
            a_sb = []
            for t in range(NT):
                ps = pst()
                for kt in range(DT):
                    nc.tensor.matmul(ps[:, 0:D], attnT[kt][:, t * P:(t + 1) * P], wo_sb[kt],
                                     start=(kt == 0), stop=(kt == DT - 1))
                x_t = XP.tile([P, D], F32, tag="x2", name="x2", bufs=2)
                nc.sync.dma_start(out=x_t, in_=x_d.ap()[t * P:(t + 1) * P, :])
                a_t = APL.tile([P, D], F32, tag="a", name="a")
                nc.vector.scalar_tensor_tensor(out=a_t, in0=ps[:, 0:D], scalar=1.0, in1=x_t,
                                               op0=OP.mult, op1=OP.add)
                nc.vector.tensor_add(a_t, a_t, bo_b)
                a_sb.append(a_t)
                stats = TMP.tile([P, 6], F32, tag="stats")
                nc.vector.bn_stats(out=stats, in_=a_t)
                mv = TMP.tile([P, 2], F32, tag="mv")
                nc.vector.bn_aggr(out=mv, in_=stats)
                std = TMP.tile([P, 1], F32, tag="std")
                nc.scalar.activation(out=std, in_=mv[:, 1:2], func=AF.Sqrt, bias=eps_t)
                rstd = TMP.tile([P, 1], F32, tag="rstd")
                nc.vector.reciprocal(out=rstd, in_=std)
                h2 = TMP.tile([P, D], F32, tag="h1")
                nc.vector.tensor_scalar(out=h2, in0=a_t, scalar1=mv[:, 0:1], scalar2=rstd,
                                        op0=OP.subtract, op1=OP.mult)
                nc.vector.tensor_mul(h2, h2, ln2g_b)
                nc.vector.tensor_add(h2, h2, ln2b_b)
                for dt_ in range(DT):
                    tp = pst()
                    nc.tensor.transpose(tp[:, 0:P], h2[:, dt_ * P:(dt_ + 1) * P], ident)
                    nc.vector.tensor_copy(h2T[dt_][:, t * P:(t + 1) * P], tp[:, 0:P])

            # ---------------- MLP (weights reuse wA/wB slots) ----------------
            wm1_sb = {}
            for kt in range(DT):
                for mt in range(MT):
                    t = WA.tile([P, P], F32, tag="wA", name="wA")
                    nc.sync.dma_start(out=t, in_=wm1_d.ap()[kt * P:(kt + 1) * P, mt * P:(mt + 1) * P])
                    wm1_sb[kt, mt] = t
            wm2_sb = {}
            for mt in range(MT):
                t = WB.tile([P, D], F32, tag="wB", name="wB")
                nc.sync.dma_start(out=t, in_=wm2h_d.ap()[mt * P:(mt + 1) * P, :])
                wm2_sb[mt] = t

            for c in range(NMC):
                ps2 = pst()   # holds TPC token-tile outputs side by side
                for mt in range(MT):
                    ps = pst()
                    for kt in range(DT):
                        nc.tensor.matmul(ps[:, 0:SQC], wm1_sb[kt, mt],
                                         h2T[kt][:, c * SQC:(c + 1) * SQC],
                                         start=(kt == 0), stop=(kt == DT - 1))
                    pre = M1P.tile([P, SQC], F32, tag="m1pre", name="m1pre")
                    nc.vector.tensor_scalar(out=pre, in0=ps[:, 0:SQC], scalar1=bm1_sb[mt],
                                            scalar2=None, op0=OP.add)
                    s = M1P.tile([P, SQC], F32, tag="m1s", name="m1s")
                    nc.gpsimd.tensor_mul(s, pre, pre)
                    nc.gpsimd.tensor_scalar(out=s, in0=s, scalar1=0.044715, scalar2=1.0,
                                            op0=OP.mult, op1=OP.add)
                    nc.gpsimd.tensor_mul(s, s, pre)
                    nc.scalar.activation(out=s, in_=s, func=AF.Tanh, scale=GELU_C)
                    g = M1P.tile([P, SQC], F32, tag="m1g", name="m1g")
                    nc.vector.scalar_tensor_tensor(out=g, in0=s, scalar=1.0, in1=pre,
                                                   op0=OP.add, op1=OP.mult)
                    for tr in range(TPC):
                        nc.tensor.matmul(ps2[:, tr * 512:tr * 512 + D],
                                         g[:, tr * P:(tr + 1) * P], wm2_sb[mt],
                                         start=(mt == 0), stop=(mt == MT - 1))
                for tr in range(TPC):
                    t = c * TPC + tr
                    f1 = TMP.tile([P, D], F32, tag="f1", bufs=2)
                    nc.vector.scalar_tensor_tensor(out=f1, in0=ps2[:, tr * 512:tr * 512 + D],
                                                   scalar=1.0, in1=a_sb[t],
                                                   op0=OP.mult, op1=OP.add)
                    nc.vector.tensor_add(f1, f1, bm2_b)
                    x_t = XP.tile([P, D], F32, tag="x3", name="x3", bufs=2)
                    nc.sync.dma_start(out=x_t, in_=x_d.ap()[t * P:(t + 1) * P, :])
                    f2 = TMP.tile([P, D], F32, tag="f2", bufs=2)
                    nc.vector.scalar_tensor_tensor(out=f2, in0=f1, scalar=wsel[:, t:t + 1],
                                                   in1=x_t, op0=OP.mult, op1=OP.add)
                    nc.sync.dma_start(out=out_d.ap()[t * P:(t + 1) * P, :], in_=f2)

    nc.compile()
    return nc


def marshal_inputs(cfg, inputs):
    """Build per-core in_maps from full inputs (numpy, fp32)."""
    S, D, H, MLP_, KH, B = (cfg[k] for k in ("S", "D", "H", "MLP", "KH", "B"))
    P = 128
    KSLICE = S * D // B
    KT = KSLICE // P
    W1COLS = B + KH
    NT = S // P

    f = lambda k: np.asarray(inputs[k], dtype=np.float32)
    x = f("x")
    amask = f("attention_mask")
    kp_w1 = f("kp_w1")
    x_flat = x.reshape(B, S * D)

    shared = dict(
        wp_w_row=f("wp_w").reshape(1, D),
        wp_b=f("wp_b").reshape(1, 1),
        kp_b1_row=f("kp_b1").reshape(1, KH),
        kp_w2_row=f("kp_w2").reshape(1, KH),
        kp_b2=f("kp_b2").reshape(1, 1),
        ln1_g_row=f("ln1_g").reshape(1, D),
        ln1_b_row=f("ln1_b").reshape(1, D),
        wqk=np.ascontiguousarray(f("wqkv")[:, :2 * D]),
        bqk_col=f("bqkv")[:2 * D].reshape(2 * D, 1),
        wv=np.ascontiguousarray(f("wqkv")[:, 2 * D:]),
        bv_row=f("bqkv")[2 * D:].reshape(1, D),
        wo=f("wo"),
        bo_row=f("bo").reshape(1, D),
        ln2_g_row=f("ln2_g").reshape(1, D),
        ln2_b_row=f("ln2_b").reshape(1, D),
        wm1=f("wm1"),
        bm1_col=f("bm1").reshape(MLP_, 1),
        wm2_half=0.5 * f("wm2"),
        bm2_row=f("bm2").reshape(1, D),
    )
    HD = D // H
    HPT = P // HD
    for ht in range(D // P):
        E = np.zeros((H, P), np.float32)
        for hr in range(HPT):
            h = ht * HPT + hr
            if h < H:
                E[h, hr * HD:(hr + 1) * HD] = 1.0
        shared[f"Emat{ht}"] = E
    shared["iota_row"] = np.arange(1, S + 1, dtype=np.float32).reshape(4, S // 4)
    in_maps = []
    for i in range(B):
        sl = slice(i * KSLICE, (i + 1) * KSLICE)
        xr = np.ascontiguousarray(x_flat[:, sl].T)             # [KSLICE, B]
        w1s = kp_w1[sl]                                        # [KSLICE, KH]
        w1x = np.concatenate(
            [xr.reshape(KT, P, B), w1s.reshape(KT, P, KH)], axis=2
        )
        w1x = np.ascontiguousarray(w1x.transpose(1, 0, 2).reshape(P, KT * W1COLS))
        m = dict(shared)
        m["x_own"] = np.ascontiguousarray(x[i])
        m["w1x"] = w1x
        m["amask_t"] = np.ascontiguousarray(amask[i, 0, 0].reshape(NT, P).T)
        in_maps.append(m)
    return in_maps


_NC_CACHE = {}


def _get_nc(cfg_key):
    if cfg_key not in _NC_CACHE:
        _NC_CACHE[cfg_key] = build_bass(FULL_CFG)
    return _NC_CACHE[cfg_key]


def run(inputs, trace=False, **kw):
    from concourse.bass_utils import run_bass_kernel_spmd

    cfg = FULL_CFG
    nc = _get_nc("full")
    in_maps = marshal_inputs(cfg, inputs)
    res = run_bass_kernel_spmd(nc, in_maps, list(range(cfg["B"])), trace=trace, **kw)
    out = np.stack([res.results[i]["out"] for i in range(cfg["B"])], axis=0)
    return out.astype(np.float32), res


def kernel(**inputs):
    return run(inputs)[0]


# revision 20
# speedup vs baseline: 1.3517x; 1.3517x over previous
"""Trainium2 Bass kernel for routed-token transformer block (moe_routing).

Strategy (8 NeuronCores):
  - Batch-parallel transformer block: core i owns sample i (B=8).
  - The dominant cost is the router k_predictor matvec x_flat @ kp_w1 with
    kp_w1 [S*D, 512] = 1GB fp32. Its contraction dim is sharded 8-ways:
    core i reads rows [i*65536, (i+1)*65536) once (128MB/core), computes
    partial sums for ALL samples [8, 512], and a ReduceScatter(add) hands
    core i the finished row for its own sample (16KB collective).
  - Attention runs in transposed-activation layout (features on partitions)
    so q/k slices feed matmuls directly. The key mask enters as a
    per-key scale on an augmented V (exp(score+m) == exp(score)*exp(m)),
    so the whole scores+exp pipeline is independent of the router and
    overlaps the kp_w1 DMA stream. An extra ones-column in the augmented V
    yields softmax denominators for free.
"""

import ml_dtypes
import numpy as np

import concourse.bass as bass
import concourse.bacc as bacc
import concourse.mybir as mybir
import concourse.tile as tile
from concourse.masks import make_identity

F32 = mybir.dt.float32
BF16 = mybir.dt.bfloat16
AF = mybir.ActivationFunctionType
OP = mybir.AluOpType

FULL_CFG = dict(S=2048, D=256, H=8, MLP=1024, KH=512, B=8)
GELU_C = 0.7978845608028654  # sqrt(2/pi)
NEG_EPS = 1e-30


def _bcast_ap(handle, p):
    """DRAM AP broadcast over p partitions (partition step 0)."""
    ap = handle.ap()
    return bass.AP(tensor=ap.tensor, offset=ap.offset,
                   ap=[[0, p]] + [list(x) for x in ap.ap[1:]])


def build_bass(cfg):
    S, D, H, MLP_, KH, B = (cfg[k] for k in ("S", "D", "H", "MLP", "KH", "B"))
    P = 128
    HD = D // H
    assert HD == 32
    NT = S // P                      # token tiles
    DT = D // P                      # feature tiles of D
    KSLICE = S * D // B              # router contraction rows per core
    KT = KSLICE // P                 # router K-tiles
    HPT = P // HD                    # heads per 128-feature tile
    CH = min(1024, S)                # attention sq chunk (psum free size)
    NSQC = S // CH
    MMN = min(512, CH)               # matmul free-dim piece
    NPC = CH // MMN
    SQC = min(256, S)                # mlp sq chunk
    NMC = S // SQC
    TPC = SQC // P                   # token tiles per mlp chunk
    MT = MLP_ // P                   # mlp feature tiles
    QKM = 2 * D // P                 # q+k feature tiles
    W1COLS = B + KH
    NQT = (H + 2) // 3               # q/k head-tiles (3 heads per tile @ bases 0/32/64)
    PSW = max(CH, TPC * 512, 512)    # shared psum slot width (bank-aligned tr slots)

    nc = bacc.Bacc()

    # ---------------- DRAM parameters ----------------
    dp = lambda name, shape: nc.declare_dram_parameter(name, list(shape), F32, isOutput=False)
    x_d = dp("x_own", (S, D))
    w1x_d = nc.declare_dram_parameter("w1x", [P, KT * W1COLS], BF16, isOutput=False)
    amask_d = dp("amask_t", (P, NT))
    wpw_d = dp("wp_w_row", (1, D))
    wpb_d = dp("wp_b", (1, 1))
    kpb1_d = dp("kp_b1_row", (1, KH))
    kpw2_d = dp("kp_w2_row", (1, KH))
    kpb2_d = dp("kp_b2", (1, 1))
    ln1g_d = dp("ln1_g_row", (1, D))
    ln1b_d = dp("ln1_b_row", (1, D))
    wqk_d = dp("wqk", (D, 2 * D))
    bqk_d = dp("bqk_col", (2 * D, 1))
    wv_d = dp("wv", (D, D))
    bv_d = dp("bv_row", (1, D))
    wo_d = dp("wo", (D, D))
    bo_d = dp("bo_row", (1, D))
    ln2g_d = dp("ln2_g_row", (1, D))
    ln2b_d = dp("ln2_b_row", (1, D))
    wm1_d = dp("wm1", (D, MLP_))
    bm1_d = dp("bm1_col", (MLP_, 1))
    wm2h_d = dp("wm2_half", (MLP_, D))   # pre-scaled by 0.5 on host (gelu's 0.5)
    bm2_d = dp("bm2_row", (1, D))
    E_d = [dp(f"Emat{ht}", (H, P)) for ht in range(D // P)]
    iota_d = dp("iota_row", (4, S // 4))
    out_d = nc.declare_dram_parameter("out", [S, D], F32, isOutput=True)

    rs_in = nc.dram_tensor("rs_in", [B, KH], F32)
    rs_out = nc.dram_tensor("rs_out", [1, KH], F32)

    with tile.TileContext(nc) as tc:
        with (
            tc.tile_pool(name="singles", bufs=1) as SG,
            tc.tile_pool(name="wA", bufs=max(DT * MT, DT * QKM)) as WA,
            tc.tile_pool(name="wB", bufs=max(MT, 2 * DT)) as WB,
            tc.tile_pool(name="w1chunk", bufs=8) as W1P,
            tc.tile_pool(name="xin", bufs=3) as XP,
            tc.tile_pool(name="tmp", bufs=3) as TMP,
            tc.tile_pool(name="hT", bufs=DT) as HTP,
            tc.tile_pool(name="qk", bufs=1) as QKP,
            tc.tile_pool(name="vaug", bufs=NT) as VAP,
            tc.tile_pool(name="expT", bufs=2) as EXP,
            tc.tile_pool(name="attnT", bufs=1) as ATP,
            tc.tile_pool(name="m1", bufs=2) as M1P,
            tc.tile_pool(name="acts", bufs=NT) as APL,
            tc.tile_pool(name="small", bufs=1) as SM,
            tc.tile_pool(name="ps", bufs=4, space="PSUM") as PS,
        ):
            pst = lambda pp=P: PS.tile([pp, PSW], F32, tag="ps", name="ps")
            # ---------------- constants ----------------
            ident = SG.tile([P, P], F32, tag="ident")
            make_identity(nc, ident)
            eps_t = SG.tile([P, 1], F32, tag="eps")
            nc.vector.memset(eps_t, 1e-5)

            def bload(handle, tag):
                t = SG.tile([P, handle.ap().ap[-1][1]], F32, tag=tag, name=tag)
                nc.sync.dma_start(out=t, in_=_bcast_ap(handle, P))
                return t

            ln1g_b = bload(ln1g_d, "ln1g")
            ln1b_b = bload(ln1b_d, "ln1b")
            ln2g_b = bload(ln2g_d, "ln2g")
            ln2b_b = bload(ln2b_d, "ln2b")
            wpw_b = bload(wpw_d, "wpw")
            wpb_b = bload(wpb_d, "wpb")
            bv_b = bload(bv_d, "bv")
            bo_b = bload(bo_d, "bo")
            bm2_b = bload(bm2_d, "bm2")

            bqk_sb = []
            for mt in range(QKM):
                t = SG.tile([P, 1], F32, tag=f"bqk{mt}", name=f"bqk{mt}")
                nc.sync.dma_start(out=t, in_=bqk_d.ap()[mt * P:(mt + 1) * P, :])
                bqk_sb.append(t)
            bm1_sb = []
            for mt in range(MT):
                t = SG.tile([P, 1], F32, tag=f"bm1{mt}", name=f"bm1{mt}")
                nc.sync.dma_start(out=t, in_=bm1_d.ap()[mt * P:(mt + 1) * P, :])
                bm1_sb.append(t)
            kpb1_sb = SG.tile([1, KH], F32, tag="kpb1")
            nc.sync.dma_start(out=kpb1_sb, in_=kpb1_d.ap())
            kpw2_sb = SG.tile([1, KH], F32, tag="kpw2")
            nc.sync.dma_start(out=kpw2_sb, in_=kpw2_d.ap())
            kpb2_sb = SG.tile([1, 1], F32, tag="kpb2")
            nc.sync.dma_start(out=kpb2_sb, in_=kpb2_d.ap())
            ones_row = SG.tile([1, P], F32, tag="ones_row")
            nc.vector.memset(ones_row, 1.0)
            E_sb = []
            for ht in range(DT):
                e = SG.tile([H, P], F32, tag=f"E{ht}", name=f"E{ht}")
                nc.sync.dma_start(out=e, in_=E_d[ht].ap())
                E_sb.append(e)
            amask_sb = SG.tile([P, NT], F32, tag="amask")
            nc.sync.dma_start(out=amask_sb, in_=amask_d.ap())
            iota_sb = SG.tile([4, S // 4], F32, tag="iota")
            nc.sync.dma_start(out=iota_sb, in_=iota_d.ap())
            ones4 = SG.tile([4, 1], F32, tag="ones4")
            nc.vector.memset(ones4, 1.0)

            # ---------------- block weights (qkv/wo; mlp loaded later in same slots) ----
            wqk_sb = {}
            for kt in range(DT):
                for mt in range(QKM):
                    t = WA.tile([P, P], F32, tag="wA", name="wA")
                    nc.sync.dma_start(out=t, in_=wqk_d.ap()[kt * P:(kt + 1) * P, mt * P:(mt + 1) * P])
                    wqk_sb[kt, mt] = t
            wv_sb = {}
            wo_sb = {}
            for kt in range(DT):
                t = WB.tile([P, D], F32, tag="wB", name="wB")
                nc.sync.dma_start(out=t, in_=wv_d.ap()[kt * P:(kt + 1) * P, :])
                wv_sb[kt] = t
                t2 = WB.tile([P, D], F32, tag="wB", name="wB")
                nc.sync.dma_start(out=t2, in_=wo_d.ap()[kt * P:(kt + 1) * P, :])
                wo_sb[kt] = t2

            # ---------------- router stream ----------------
            ps_router = pst(B)
            for kt in range(KT):
                chunk = W1P.tile([P, W1COLS], BF16, tag="w1c", name="w1c")
                nc.sync.dma_start(out=chunk, in_=w1x_d.ap()[:, kt * W1COLS:(kt + 1) * W1COLS])
                nc.tensor.matmul(ps_router[:, 0:KH], chunk[:, 0:B], chunk[:, B:W1COLS],
                                 start=(kt == 0), stop=(kt == KT - 1))

            # ---------------- LN1 + h1T + token weights ----------------
            h1T = [HTP.tile([P, S], F32, tag="hT", name=f"h1T{i}") for i in range(DT)]
            weights_sb = SM.tile([P, NT], F32, tag="weights")
            for t in range(NT):
                x_t = XP.tile([P, D], F32, tag="x1", name="x1")
                nc.sync.dma_start(out=x_t, in_=x_d.ap()[t * P:(t + 1) * P, :])
                stats = TMP.tile([P, 6], F32, tag="stats")
                nc.vector.bn_stats(out=stats, in_=x_t)
                mv = TMP.tile([P, 2], F32, tag="mv")
                nc.vector.bn_aggr(out=mv, in_=stats)
                std = TMP.tile([P, 1], F32, tag="std")
                nc.scalar.activation(out=std, in_=mv[:, 1:2], func=AF.Sqrt, bias=eps_t)
                rstd = TMP.tile([P, 1], F32, tag="rstd")
                nc.vector.reciprocal(out=rstd, in_=std)
                h1 = TMP.tile([P, D], F32, tag="h1")
                nc.vector.tensor_scalar(out=h1, in0=x_t, scalar1=mv[:, 0:1], scalar2=rstd,
                                        op0=OP.subtract, op1=OP.mult)
                nc.vector.tensor_mul(h1, h1, ln1g_b)
                nc.vector.tensor_add(h1, h1, ln1b_b)
                scr = TMP.tile([P, D], F32, tag="scr", bufs=1)
                nc.vector.scalar_tensor_tensor(out=scr, in0=x_t, scalar=1.0, in1=wpw_b,
                                               op0=OP.mult, op1=OP.mult,
                                               accum_out=weights_sb[:, t:t + 1])
                for dt_ in range(DT):
                    tp = pst()
                    nc.tensor.transpose(tp[:, 0:P], h1[:, dt_ * P:(dt_ + 1) * P], ident)
                    nc.vector.tensor_copy(h1T[dt_][:, t * P:(t + 1) * P], tp[:, 0:P])
            nc.vector.tensor_scalar_add(weights_sb, weights_sb, wpb_b[:, 0:1])
            expamask = SM.tile([P, NT], F32, tag="expamask")
            nc.scalar.activation(out=expamask, in_=amask_sb, func=AF.Exp)

            # ---------------- qkT (heads packed 3-per-tile @ bases 0/32/64) ----------
            qh_sb = [QKP.tile([P, S], F32, tag=f"qh{j}", name=f"qh{j}") for j in range(NQT)]
            kh_sb = [QKP.tile([P, S], F32, tag=f"kh{j}", name=f"kh{j}") for j in range(NQT)]

            def head_slice(tiles, h):
                b = 32 * (h % 3)
                return tiles[h // 3][b:b + HD, :]

            QN = min(512, S)
            for mt in range(QKM):
                for ncn in range(S // QN):
                    ps = pst()
                    for kt in range(DT):
                        nc.tensor.matmul(ps[:, 0:QN], wqk_sb[kt, mt],
                                         h1T[kt][:, ncn * QN:(ncn + 1) * QN],
                                         start=(kt == 0), stop=(kt == DT - 1))
                    for g in range(HPT):
                        h = (mt % DT) * HPT + g
                        dst = head_slice(kh_sb if mt >= DT else qh_sb, h)
                        nc.vector.tensor_scalar(
                            out=dst[:, ncn * QN:(ncn + 1) * QN],
                            in0=ps[g * HD:(g + 1) * HD, 0:QN],
                            scalar1=bqk_sb[mt][g * HD:(g + 1) * HD, :],
                            scalar2=None, op0=OP.add)

            # ---------------- V (token-major) + ones col ----------------
            v_aug = []
            for t in range(NT):
                ps = pst()
                for kt in range(DT):
                    nc.tensor.matmul(ps[:, 0:D], h1T[kt][:, t * P:(t + 1) * P], wv_sb[kt],
                                     start=(kt == 0), stop=(kt == DT - 1))
                va = VAP.tile([P, H, HD + 1], F32, tag="vaug", name="vaug")
                nc.vector.tensor_add(va[:, :, 0:HD],
                                     ps[:, 0:D].rearrange("p (h d) -> p h d", h=H),
                                     bv_b.rearrange("p (h d) -> p h d", h=H))
                nc.vector.memset(va[:, :, HD:HD + 1], 1.0)
                v_aug.append(va)

            # ---------------- router epilogue -> k -> sel -> m01/wsel ----------------
            r8 = SM.tile([B, KH], F32, tag="r8")
            nc.vector.tensor_copy(r8, ps_router[:, 0:KH])
            nc.sync.dma_start(out=rs_in.ap(), in_=r8)
            nc.gpsimd.collective_compute(
                "ReduceScatter", OP.add,
                ins=[rs_in.ap()], outs=[rs_out.ap()],
                replica_groups=[list(range(B))],
            )
            klr = SM.tile([1, KH], F32, tag="klr")
            nc.sync.dma_start(out=klr, in_=rs_out.ap())
            nc.vector.tensor_add(klr, klr, kpb1_sb)
            nc.vector.scalar_tensor_tensor(out=klr, in0=klr, scalar=0.01, in1=klr,
                                            op0=OP.mult, op1=OP.max)   # leaky_relu
            scr2 = SM.tile([1, KH], F32, tag="scr2")
            kl2 = SM.tile([1, 1], F32, tag="kl2")
            nc.vector.scalar_tensor_tensor(out=scr2, in0=klr, scalar=1.0, in1=kpw2_sb,
                                           op0=OP.mult, op1=OP.mult, accum_out=kl2)
            nc.vector.tensor_add(kl2, kl2, kpb2_sb)
            sg = SM.tile([1, 1], F32, tag="sg")
            nc.scalar.activation(out=sg, in_=kl2, func=AF.Exp, scale=-1.0)
            nc.vector.tensor_scalar_add(sg, sg, 1.0)
            nc.vector.reciprocal(sg, sg)
            kv = SM.tile([1, 1], F32, tag="kv")
            nc.vector.tensor_scalar(out=kv, in0=sg, scalar1=float(S), scalar2=1.0,
                                    op0=OP.mult, op1=OP.max)
            nc.vector.tensor_scalar_min(kv, kv, float(S))
            # k = floor(clip(sig*S,1,S)) == number of j in [1,S] with j <= v
            psv = pst()
            nc.tensor.matmul(psv[0:4, 0:1], ones_row[:, 0:4], kv,
                             start=True, stop=True)
            vb4 = SM.tile([4, 1], F32, tag="vb4")
            nc.vector.tensor_copy(vb4, psv[0:4, 0:1])
            kcmp = SM.tile([4, S // 4], F32, tag="kcmp")
            nc.vector.tensor_single_scalar(out=kcmp, in_=iota_sb, scalar=vb4,
                                           op=OP.is_le)
            cnt4 = SM.tile([4, 1], F32, tag="cnt4")
            nc.vector.tensor_reduce(out=cnt4, in_=kcmp, axis=mybir.AxisListType.X,
                                    op=OP.add)
            psc = pst()
            nc.tensor.matmul(psc[0:1, 0:1], ones4, cnt4, start=True, stop=True)
            nc.vector.tensor_copy(kv, psc[0:1, 0:1])
            psk = pst()
            nc.tensor.matmul(psk[:, 0:1], ones_row, kv, start=True, stop=True)
            kb = SM.tile([P, 1], F32, tag="kb")
            nc.vector.tensor_copy(kb, psk[:, 0:1])
            sel01 = SM.tile([P, NT], F32, tag="sel01")
            nc.vector.tensor_single_scalar(out=sel01, in_=weights_sb, scalar=kb[:, 0:1], op=OP.is_gt)
            m01 = SM.tile([P, NT], F32, tag="m01")
            nc.vector.tensor_mul(m01, sel01, expamask)
            wsel = SM.tile([P, NT], F32, tag="wsel")
            nc.vector.tensor_mul(wsel, weights_sb, sel01)
            for t in range(NT):
                nc.vector.tensor_scalar_mul(v_aug[t], v_aug[t], m01[:, t:t + 1])

            # ---------------- attention ----------------
            scale = 1.0 / float(np.sqrt(HD))
            attnT = [ATP.tile([P, S], F32, tag=f"attnT{ht}", name=f"attnT{ht}") for ht in range(DT)]
            den_sb = SM.tile([H, S], F32, tag="den")
            for h in range(H):
                ht, hr = divmod(h, HPT)
                qT = head_slice(qh_sb, h)
                kT = head_slice(kh_sb, h)
                for sqc in range(NSQC):
                    pv = pst(HD + 1)
                    for skt in range(NT):
                        ps_s = pst()
                        for j in range(NPC):
                            nc.tensor.matmul(ps_s[:, j * MMN:(j + 1) * MMN],
                                             kT[:, skt * P:(skt + 1) * P],
                                             qT[:, sqc * CH + j * MMN:sqc * CH + (j + 1) * MMN],
                                             start=True, stop=True)
                        et = EXP.tile([P, CH], F32, tag="expT", name="expT")
                        nc.scalar.activation(out=et, in_=ps_s[:, 0:CH], func=AF.Exp, scale=scale)
                        for j in range(NPC):
                            nc.tensor.matmul(pv[:, j * MMN:(j + 1) * MMN],
                                             v_aug[skt][:, h, :],
                                             et[:, j * MMN:(j + 1) * MMN],
                                             start=(skt == 0), stop=(skt == NT - 1))
                    nc.vector.tensor_copy(attnT[ht][hr * HD:(hr + 1) * HD, sqc * CH:(sqc + 1) * CH],
                                          pv[0:HD, 0:CH])
                    dstg = TMP.tile([1, CH], F32, tag="dstg", bufs=2)
                    nc.vector.tensor_copy(dstg, pv[HD:HD + 1, 0:CH])
                    nc.sync.dma_start(out=den_sb[h:h + 1, sqc * CH:(sqc + 1) * CH], in_=dstg)
            nc.vector.tensor_scalar_add(den_sb, den_sb, NEG_EPS)
            nc.vector.reciprocal(den_sb, den_sb)
            for ht in range(DT):
                for sqc in range(NSQC):
                    psb_ = pst()
                    for j in range(NPC):
                        nc.tensor.matmul(psb_[:, j * MMN:(j + 1) * MMN], E_sb[ht],
                                         den_sb[:, sqc * CH + j * MMN:sqc * CH + (j + 1) * MMN],
                                         start=True, stop=True)
                    sl = attnT[ht][:, sqc * CH:(sqc + 1) * CH]
                    nc.vector.tensor_mul(sl, sl, psb_[:, 0:CH])

            # ---------------- a = x + attn@wo + bo ; LN2 -> h2T ----------------
            h2T = [HTP.tile([P, S], F32, tag="hT", name=f"h2T{i}") for i in range(DT)]
            a_sb = []
            for t in range(NT):
                ps = pst()
                for kt in range(DT):
                    nc.tensor.matmul(ps[:, 0:D], attnT[kt][:, t * P:(t + 1) * P], wo_sb[kt],
                                     start=(kt == 0), stop=(kt == DT - 1))
                x_t = XP.tile([P, D], F32, tag="x2", name="x2", bufs=2)
                nc.sync.dma_start(out=x_t, in_=x_d.ap()[t * P:(t + 1) * P, :])
                a_t = APL.tile([P, D], F32, tag="a", name="a")
                nc.vector.scalar_tensor_tensor(out=a_t, in0=ps[:, 0:D], scalar=1.0, in1=x_t,
                                               op0=OP.mult, op1=OP.add)
                nc.vector.tensor_add(a_t, a_t, bo_b)
                a_sb.append(a_t)
                stats = TMP.tile([P, 6], F32, tag="stats")
                nc.vector.bn_stats(out=stats, in_=a_t)
                mv = TMP.tile([P, 2], F32, tag="mv")
                nc.vector.bn_aggr(out=mv, in_=stats)
                std = TMP.tile([P, 1], F32, tag="std")
                nc.scalar.activation(out=std, in_=mv[:, 1:2], func=AF.Sqrt, bias=eps_t)
                rstd = TMP.tile([P, 1], F32, tag="rstd")
                nc.vector.reciprocal(out=rstd, in_=std)
                h2 = TMP.tile([P, D], F32, tag="h1")
                nc.vector.tensor_scalar(out=h2, in0=a_t, scalar1=mv[:, 0:1], scalar2=rstd,
                                        op0=OP.subtract, op1=OP.mult)
                nc.vector.tensor_mul(h2, h2, ln2g_b)
                nc.vector.tensor_add(h2, h2, ln2b_b)
                for dt_ in range(DT):
                    tp = pst()
                    nc.tensor.transpose(tp[:, 0:P], h2[:, dt_ * P:(dt_ + 1) * P], ident)
                    nc.vector.tensor_copy(h2T[dt_][:, t * P:(t + 1) * P], tp[:, 0:P])

            # ---------------- MLP (weights reuse wA/wB slots) ----------------
            wm1_sb = {}
            for kt in range(DT):
                for mt in range(MT):
                    t = WA.tile([P, P], F32, tag="wA", name="wA")
                    nc.sync.dma_start(out=t, in_=wm1_d.ap()[kt * P:(kt + 1) * P, mt * P:(mt + 1) * P])
                    wm1_sb[kt, mt] = t
            wm2_sb = {}
            for mt in range(MT):
                t = WB.tile([P, D], F32, tag="wB", name="wB")
                nc.sync.dma_start(out=t, in_=wm2h_d.ap()[mt * P:(mt + 1) * P, :])
                wm2_sb[mt] = t

            for c in range(NMC):
                ps2 = pst()   # holds TPC token-tile outputs side by side
                for mt in range(MT):
                    ps = pst()
                    for kt in range(DT):
                        nc.tensor.matmul(ps[:, 0:SQC], wm1_sb[kt, mt],
                                         h2T[kt][:, c * SQC:(c + 1) * SQC],
                                         start=(kt == 0), stop=(kt == DT - 1))
                    pre = M1P.tile([P, SQC], F32, tag="m1pre", name="m1pre")
                    nc.vector.tensor_scalar(out=pre, in0=ps[:, 0:SQC], scalar1=bm1_sb[mt],
                                            scalar2=None, op0=OP.add)
                    s = M1P.tile([P, SQC], F32, tag="m1s", name="m1s")
                    nc.gpsimd.tensor_mul(s, pre, pre)
                    nc.gpsimd.tensor_scalar(out=s, in0=s, scalar1=0.044715, scalar2=1.0,
                                            op0=OP.mult, op1=OP.add)
                    nc.gpsimd.tensor_mul(s, s, pre)
                    nc.scalar.activation(out=s, in_=s, func=AF.Tanh, scale=GELU_C)
                    g = M1P.tile([P, SQC], F32, tag="m1g", name="m1g")
                    nc.vector.scalar_tensor_tensor(out=g, in0=s, scalar=1.0, in1=pre,
                                                   op0=OP.add, op1=OP.mult)
                    for tr in range(TPC):
                        nc.tensor.matmul(ps2[:, tr * 512:tr * 512 + D],
                                         g[:, tr * P:(tr + 1) * P], wm2_sb[mt],
                                         start=(mt == 0), stop=(mt == MT - 1))
                for tr in range(TPC):
                    t = c * TPC + tr
                    f1 = TMP.tile([P, D], F32, tag="f1", bufs=2)
                    nc.vector.scalar_tensor_tensor(out=f1, in0=ps2[:, tr * 512:tr * 512 + D],
                                                   scalar=1.0, in1=a_sb[t],
                                                   op0=OP.mult, op1=OP.add)
                    nc.vector.tensor_add(f1, f1, bm2_b)
                    x_t = XP.tile([P, D], F32, tag="x3", name="x3", bufs=2)
                    nc.sync.dma_start(out=x_t, in_=x_d.ap()[t * P:(t + 1) * P, :])
                    f2 = TMP.tile([P, D], F32, tag="f2", bufs=2)
                    nc.vector.scalar_tensor_tensor(out=f2, in0=f1, scalar=wsel[:, t:t + 1],
                                                   in1=x_t, op0=OP.mult, op1=OP.add)
                    nc.sync.dma_start(out=out_d.ap()[t * P:(t + 1) * P, :], in_=f2)

    nc.compile()
    return nc


def marshal_inputs(cfg, inputs):
    """Build per-core in_maps from full inputs (numpy, fp32)."""
    S, D, H, MLP_, KH, B = (cfg[k] for k in ("S", "D", "H", "MLP", "KH", "B"))
    P = 128
    KSLICE = S * D // B
    KT = KSLICE // P
    W1COLS = B + KH
    NT = S // P

    f = lambda k: np.asarray(inputs[k], dtype=np.float32)
    x = f("x")
    amask = f("attention_mask")
    kp_w1 = f("kp_w1")
    x_flat = x.reshape(B, S * D)

    shared = dict(
        wp_w_row=f("wp_w").reshape(1, D),
        wp_b=f("wp_b").reshape(1, 1),
        kp_b1_row=f("kp_b1").reshape(1, KH),
        kp_w2_row=f("kp_w2").reshape(1, KH),
        kp_b2=f("kp_b2").reshape(1, 1),
        ln1_g_row=f("ln1_g").reshape(1, D),
        ln1_b_row=f("ln1_b").reshape(1, D),
        wqk=np.ascontiguousarray(f("wqkv")[:, :2 * D]),
        bqk_col=f("bqkv")[:2 * D].reshape(2 * D, 1),
        wv=np.ascontiguousarray(f("wqkv")[:, 2 * D:]),
        bv_row=f("bqkv")[2 * D:].reshape(1, D),
        wo=f("wo"),
        bo_row=f("bo").reshape(1, D),
        ln2_g_row=f("ln2_g").reshape(1, D),
        ln2_b_row=f("ln2_b").reshape(1, D),
        wm1=f("wm1"),
        bm1_col=f("bm1").reshape(MLP_, 1),
        wm2_half=0.5 * f("wm2"),
        bm2_row=f("bm2").reshape(1, D),
    )
    HD = D // H
    HPT = P // HD
    for ht in range(D // P):
        E = np.zeros((H, P), np.float32)
        for hr in range(HPT):
            h = ht * HPT + hr
            if h < H:
                E[h, hr * HD:(hr + 1) * HD] = 1.0
        shared[f"Emat{ht}"] = E
    shared["iota_row"] = np.arange(1, S + 1, dtype=np.float32).reshape(4, S // 4)
    in_maps = []
    for i in range(B):
        sl = slice(i * KSLICE, (i + 1) * KSLICE)
        xr = np.ascontiguousarray(x_flat[:, sl].T)             # [KSLICE, B]
        w1s = kp_w1[sl]                                        # [KSLICE, KH]
        w1x = np.concatenate(
            [xr.reshape(KT, P, B), w1s.reshape(KT, P, KH)], axis=2
        )
        w1x = np.ascontiguousarray(
            w1x.transpose(1, 0, 2).reshape(P, KT * W1COLS)).astype(ml_dtypes.bfloat16)
        m = dict(shared)
        m["x_own"] = np.ascontiguousarray(x[i])
        m["w1x"] = w1x
        m["amask_t"] = np.ascontiguousarray(amask[i, 0, 0].reshape(NT, P).T)
        in_maps.append(m)
    return in_maps


_NC_CACHE = {}


def _get_nc(cfg_key):
    if cfg_key not in _NC_CACHE:
        _NC_CACHE[cfg_key] = build_bass(FULL_CFG)
    return _NC_CACHE[cfg_key]


def run(inputs, trace=False, **kw):
    from concourse.bass_utils import run_bass_kernel_spmd

    cfg = FULL_CFG
    nc = _get_nc("full")
    in_maps = marshal_inputs(cfg, inputs)
    res = run_bass_kernel_spmd(nc, in_maps, list(range(cfg["B"])), trace=trace, **kw)
    out = np.stack([res.results[i]["out"] for i in range(cfg["B"])], axis=0)
    return out.astype(np.float32), res


def kernel(**inputs):
    return run(inputs)[0]
